# revision 6
# baseline (speedup 1.0000x reference)
"""Trainium2 Bass kernel for nn_FFU_87625922773299 (sparse_attention).

Self-contained: accepts FULL inputs, shards over 8 NeuronCores
(core = 2*b + half; each core does one sample's attention i-half plus the
matching conv/upsample/combine output rows), returns FULL [4,2,128,128].
"""
import numpy as np

import concourse.bass as bass
import concourse.mybir as mybir
import concourse.tile as tile
from concourse import bacc
from concourse.bass_utils import run_bass_kernel_spmd

f32 = mybir.dt.float32
f32r = mybir.dt.float32r
Act = mybir.ActivationFunctionType

B, CIN, H, W = 4, 1024, 128, 128
CY, HY, WY = 2, 64, 64
N = HY * WY
ISPAN = 2112           # per-core i-range (33 y-rows * 64)
I0S = [0, 1984]
NROW = 64              # output rows per core
NSLOT = 66             # x row-slots (64 rows + top/bottom halo-or-zero)
WP = 130               # padded width (guard cols)
RG = 12                # conv row-group size
SUPERS = [(0, 1024), (1024, 1024), (2048, 64)]
NCHUNK = 8
PB = 1                 # P18 front guard
P18LEN = PB + NSLOT * WP + 1


def build_program(repeat=1):
    nc = bacc.Bacc(trn_type="TRN2")
    xsh = nc.dram_tensor("xsh", [CIN, NSLOT, WP], f32r, kind="ExternalInput")
    qfp = nc.dram_tensor("qfp", [128, ISPAN], f32r, kind="ExternalInput")
    kfp = nc.dram_tensor("kfp", [128, 32, 128], f32r, kind="ExternalInput")
    von = nc.dram_tensor("von", [128, 32, 3], f32r, kind="ExternalInput")
    wcd = nc.dram_tensor("wcd", [128, NCHUNK, 18], f32r, kind="ExternalInput")
    s18 = nc.dram_tensor("s18", [128, 9, 2], f32r, kind="ExternalInput")
    sht = nc.dram_tensor("sht", [33, 64], f32r, kind="ExternalInput")
    swt = nc.dram_tensor("swt", [64, 128], f32r, kind="ExternalInput")
    yhw = nc.dram_tensor("yhw", [33, 2, 64], f32r, kind="ExternalInput")
    gmb = nc.dram_tensor("gmb", [33, 1], f32, kind="ExternalInput")
    bna = nc.dram_tensor("bna", [2, 1], f32, kind="ExternalInput")
    bnb = nc.dram_tensor("bnb", [2, 1], f32, kind="ExternalInput")
    o = nc.dram_tensor("o", [2, NROW, 128], f32, kind="ExternalOutput")

    with tile.TileContext(nc) as tc:
        with (
            tc.tile_pool(name="const", bufs=1) as cp,
            tc.tile_pool(name="xg", bufs=1) as xgp,
            tc.tile_pool(name="big", bufs=1) as bigp,
            tc.tile_pool(name="we", bufs=2) as wep,
            tc.tile_pool(name="sm", bufs=2) as smp,
        ):
            qf_sb = cp.tile([128, ISPAN], f32r)
            nc.sync.dma_start(out=qf_sb[:], in_=qfp[:])
            kf_sb = cp.tile([128, 32, 128], f32r)
            nc.sync.dma_start(out=kf_sb[:], in_=kfp[:])
            von_sb = cp.tile([128, 32, 3], f32r)
            nc.sync.dma_start(out=von_sb[:], in_=von[:])
            wc_sb = cp.tile([128, NCHUNK, 18], f32r)
            nc.sync.dma_start(out=wc_sb[:], in_=wcd[:])
            s18_sb = cp.tile([128, 9, 2], f32r)
            nc.sync.dma_start(out=s18_sb[:], in_=s18[:])
            sht_sb = cp.tile([33, 64], f32r)
            nc.sync.dma_start(out=sht_sb[:], in_=sht[:])
            swt_sb = cp.tile([64, 128], f32r)
            nc.sync.dma_start(out=swt_sb[:], in_=swt[:])
            yhw_sb = cp.tile([33, 2, 64], f32r)
            nc.sync.dma_start(out=yhw_sb[:], in_=yhw[:])
            gmb_sb = cp.tile([33, 1], f32)
            nc.sync.dma_start(out=gmb_sb[:], in_=gmb[:])
            bna_sb = cp.tile([2, 1], f32)
            nc.sync.dma_start(out=bna_sb[:], in_=bna[:])
            bnb_sb = cp.tile([2, 1], f32)
            nc.sync.dma_start(out=bnb_sb[:], in_=bnb[:])

            p18_sb = bigp.tile([128, P18LEN], f32r)
            numer_sb = bigp.tile([3, ISPAN], f32)
            xc_sb = bigp.tile([2, NROW * 128], f32)

            # zero-fill p18 once (rows >=18 + guards stay zero forever)
            zsrc = cp.tile([128, 1], f32)
            nc.vector.memset(zsrc[:], 0.0)
            zbc = bass.AP(tensor=zsrc.tensor, offset=zsrc.offset,
                          ap=[zsrc.ap[0], [0, P18LEN]])
            nc.vector.tensor_copy(out=p18_sb[:], in_=zbc)

            def body(it):

                # ---- phase 1: attention + conv main (8 psum banks) ----
                with (
                    tc.tile_pool(name="eps", bufs=2, space="PSUM") as epsp,
                    tc.tile_pool(name="nps", bufs=1, space="PSUM") as npsp,
                    tc.tile_pool(name="cps", bufs=2, space="PSUM") as cpsp,
                ):
                    for (s0, span) in SUPERS:
                        nsub = (span + 511) // 512
                        n_ps = npsp.tile([3, span], f32, tag="numer",
                                         name=f"n_{it}_{s0}")
                        for g in range(32):
                            e_ps = epsp.tile([128, span], f32, tag="E",
                                             name=f"e_{it}_{s0}_{g}")
                            for t in range(nsub):
                                a, b2 = t * 512, min(span, (t + 1) * 512)
                                nc.tensor.matmul(
                                    e_ps[:, a:b2], kf_sb[:, g, :],
                                    qf_sb[:, s0 + a:s0 + b2],
                                    start=True, stop=True)
                            wexp = wep.tile([128, span], f32r, tag="W",
                                            name=f"w_{it}_{s0}_{g}")
                            nc.scalar.activation(wexp[:], e_ps[:], Act.Exp)
                            for t in range(nsub):
                                a, b2 = t * 512, min(span, (t + 1) * 512)
                                nc.tensor.matmul(
                                    n_ps[:, a:b2], von_sb[:, g, :], wexp[:, a:b2],
                                    start=(g == 0), stop=(g == 31))
                        nc.vector.tensor_copy(out=numer_sb[:, s0:s0 + span],
                                              in_=n_ps[:])

                    ngrp = (NROW + RG - 1) // RG
                    for gi in range(ngrp):
                        r0 = gi * RG
                        nrows = min(RG, NROW - r0)
                        n_sl = nrows + 2
                        glen = n_sl * WP
                        xg = xgp.tile([128, NCHUNK, (RG + 2) * WP], f32r,
                                      tag="xg", name=f"xg_{it}_{gi}")
                        for ch in range(NCHUNK):
                            nc.sync.dma_start(
                                out=xg[:, ch, 0:glen],
                                in_=xsh[ch * 128:(ch + 1) * 128, r0:r0 + n_sl, :]
                                .rearrange("c s w -> c (s w)"))
                        nblk = (glen + 511) // 512
                        for t in range(nblk):
                            q0 = t * 512
                            qn = min(512, glen - q0)
                            c_ps = cpsp.tile([18, 512], f32, tag="P18",
                                             name=f"c_{it}_{gi}_{t}")
                            for ch in range(NCHUNK):
                                nc.tensor.matmul(
                                    c_ps[:18, 0:qn], wc_sb[:, ch, :],
                                    xg[:, ch, q0:q0 + qn],
                                    start=(ch == 0), stop=(ch == NCHUNK - 1))
                            dst = PB + r0 * WP + q0
                            nc.vector.tensor_copy(
                                out=p18_sb[0:18, dst:dst + qn],
                                in_=c_ps[:18, 0:qn])

                # ---- phase 2: tap-sum + BN/ReLU + tail (fresh psum) ----
                with (
                    tc.tile_pool(name="tps", bufs=2, space="PSUM") as tpsp,
                    tc.tile_pool(name="dscr", bufs=1, space="DRAM") as dsp,
                ):
                    olen = NROW * WP
                    nblk = (olen + 511) // 512
                    for t in range(nblk):
                        q0 = t * 512
                        qn = min(512, olen - q0)
                        t_ps = tpsp.tile([2, 512], f32, tag="TAP",
                                         name=f"t_{it}_{t}")
                        k = 0
                        for dy in range(3):
                            for dx in range(3):
                                off = PB + q0 + dy * WP + dx - 1
                                nc.tensor.matmul(
                                    t_ps[:2, 0:qn], s18_sb[:, dy * 3 + dx, :],
                                    p18_sb[:, off:off + qn],
                                    start=(k == 0), stop=(k == 8))
                                k += 1
                        r_lo, r_hi = q0 // WP, (q0 + qn - 1) // WP
                        for r in range(r_lo, r_hi + 1):
                            a = max(q0, r * WP + 1)
                            b2 = min(q0 + qn, r * WP + 129)
                            if a >= b2 or r >= NROW:
                                continue
                            w_lo = a - (r * WP + 1)
                            nc.scalar.activation(
                                out=xc_sb[:, r * 128 + w_lo:r * 128 + w_lo + (b2 - a)],
                                in_=t_ps[:2, a - q0:b2 - q0],
                                func=Act.Relu, bias=bnb_sb[:], scale=bna_sb[:])

                    # attention numer -> h-part layout (via DRAM scratch)
                    ndr = dsp.tile([3, ISPAN], f32, tag="ndr", name=f"ndr_{it}")
                    nc.sync.dma_start(out=ndr[:], in_=numer_sb[:])
                    nm_hw = smp.tile([33, 3, 64], f32, tag="nmhw", name=f"nm_{it}")
                    nc.sync.dma_start(
                        out=nm_hw[:],
                        in_=ndr.rearrange("c (h w) -> h c w", h=33))
                    rcp = smp.tile([33, 1, 64], f32, tag="rcp", name=f"r_{it}")
                    nc.vector.reciprocal(out=rcp[:], in_=nm_hw[:, 0:1, :])
                    att = smp.tile([33, 2, 64], f32, tag="att", name=f"a_{it}")
                    for c in range(2):
                        nc.vector.tensor_mul(out=att[:, c, :],
                                             in0=nm_hw[:, c + 1, :],
                                             in1=rcp[:, 0, :])
                    y1hw = smp.tile([33, 2, 64], f32r, tag="y1hw", name=f"y1_{it}")
                    tmp = smp.tile([33, 2, 64], f32, tag="tmpy", name=f"ty_{it}")
                    nc.vector.tensor_scalar_mul(tmp[:], att[:], gmb_sb[:])
                    nc.vector.tensor_add(out=y1hw[:], in0=tmp[:], in1=yhw_sb[:])

                    # upsample y and y1 -> [w' 128, h' 64] per channel
                    ups = []
                    for (src, nm) in ((yhw_sb, "uy"), (y1hw, "uy1")):
                        for c in range(2):
                            t1p = tpsp.tile([64, 64], f32, tag="T1",
                                            name=f"t1_{it}_{nm}_{c}")
                            nc.tensor.matmul(t1p[:], src[:, c, :], sht_sb[:],
                                             start=True, stop=True)
                            t1s = smp.tile([64, 64], f32r, tag="T1s",
                                           name=f"t1s_{it}_{nm}_{c}")
                            nc.vector.tensor_copy(out=t1s[:], in_=t1p[:])
                            up = tpsp.tile([128, 64], f32, tag="UP",
                                           name=f"up_{it}_{nm}_{c}")
                            nc.tensor.matmul(up[:], swt_sb[:], t1s[:],
                                             start=True, stop=True)
                            us = smp.tile([128, 64], f32, tag=f"us{nm}{c}",
                                          name=f"us_{it}_{nm}_{c}")
                            nc.vector.tensor_copy(out=us[:], in_=up[:])
                            ups.append(us)
                    yup0, yup1, y1up0, y1up1 = ups

                    # xc [2, r*128+w] -> xcT [w 128, r, c] via scatter DMA
                    xdr = dsp.tile([2, NROW, 128], f32, tag="xdr", name=f"xdr_{it}")
                    nc.sync.dma_start(out=xdr.rearrange("c r w -> c (r w)"),
                                      in_=xc_sb[:])
                    xcT = smp.tile([128, 2, NROW], f32, tag="xcT",
                                   name=f"xct_{it}")
                    nc.sync.dma_start(out=xcT[:],
                                      in_=xdr.rearrange("c r w -> w c r"))

                    comb = smp.tile([128, 2, NROW], f32, tag="comb",
                                    name=f"cb_{it}")
                    for c, (yu, y1u) in enumerate(((yup0, y1up0),
                                                   (yup1, y1up1))):
                        nc.vector.tensor_mul(out=comb[:, c, :],
                                             in0=xcT[:, c, :], in1=y1u[:])
                        nc.vector.tensor_add(out=comb[:, c, :],
                                             in0=comb[:, c, :], in1=yu[:])
                    for c in range(2):
                        nc.sync.dma_start(out=o[c].rearrange("r w -> w r"),
                                          in_=comb[:, c, :])

            if repeat == 1:
                body(0)
            else:
                with tc.For_i(0, repeat):
                    body(0)
    nc.compile()
    return nc


# ---------------- host side ----------------

def _upsample_matrix(out_n, in_n):
    s = np.zeros((out_n, in_n), np.float32)
    pos = np.arange(out_n) * (in_n - 1.0) / (out_n - 1.0)
    lo = np.floor(pos).astype(np.int64)
    hi = np.minimum(lo + 1, in_n - 1)
    w = (pos - lo).astype(np.float32)
    s[np.arange(out_n), lo] += 1.0 - w
    s[np.arange(out_n), hi] += w
    return s


def shard_inputs(x, y, wq, bq, wk, bk, wv, bv, gamma, conv_w, conv_b,
                 bn_scale, bn_bias, bn_mean, bn_var):
    x = np.asarray(x, np.float32)
    y = np.asarray(y, np.float32)
    yf = y.reshape(B, CY, N)
    qf = np.einsum("oc,bcn->bon", np.asarray(wq, np.float32), yf) + \
        np.asarray(bq, np.float32)[None, :, None]
    kf = np.einsum("oc,bcn->bon", np.asarray(wk, np.float32), yf) + \
        np.asarray(bk, np.float32)[None, :, None]
    vf = np.einsum("oc,bcn->bon", np.asarray(wv, np.float32), yf) + \
        np.asarray(bv, np.float32)[None, :, None]

    # conv weights: [128 cin_in_chunk, chunk, (dy,dx,co)]
    cw = np.asarray(conv_w, np.float32)  # [2, 1024, 3, 3]
    wc = np.zeros((128, NCHUNK, 18), np.float32)
    for ch in range(NCHUNK):
        blk = cw[:, ch * 128:(ch + 1) * 128, :, :]  # [2,128,3,3]
        wc[:, ch, :] = blk.transpose(1, 2, 3, 0).reshape(128, 18)

    s18 = np.zeros((128, 9, 2), np.float32)
    for dy in range(3):
        for dx in range(3):
            for co in range(2):
                s18[dy * 6 + dx * 2 + co, dy * 3 + dx, co] = 1.0

    sh_full = _upsample_matrix(H, HY)     # [128, 64]
    sw = _upsample_matrix(W, WY)          # [128, 64]
    swt = sw.T.copy()                     # [64, 128] SwT[w, w'] = Sw[w', w]

    inv = np.asarray(bn_scale, np.float32) / np.sqrt(
        np.asarray(bn_var, np.float32) + 1e-5)
    bna = inv.reshape(2, 1)
    bnb = ((np.asarray(conv_b, np.float32) - np.asarray(bn_mean, np.float32))
           * inv + np.asarray(bn_bias, np.float32)).reshape(2, 1)
    gval = float(np.asarray(gamma).reshape(-1)[0])

    in_maps = []
    for core in range(8):
        b, half = core // 2, core % 2
        i0 = I0S[half]
        r_base = half * 64

        xsh = np.zeros((CIN, NSLOT, WP), np.float32)
        lo_row = r_base - 1
        for s in range(NSLOT):
            row = lo_row + s
            if 0 <= row < H:
                xsh[:, s, 1:129] = x[b, :, row, :]

        qfp = np.zeros((128, ISPAN), np.float32)
        qfp[0:2] = qf[b, :, i0:i0 + ISPAN]
        kfp = np.zeros((128, 32, 128), np.float32)
        kfp[0:2] = kf[b].reshape(2, 32, 128)
        von = np.zeros((128, 32, 3), np.float32)
        von[:, :, 0] = 1.0
        von[:, :, 1] = vf[b, 0].reshape(32, 128).T
        von[:, :, 2] = vf[b, 1].reshape(32, 128).T

        hrow0 = 31 * half
        sht = sh_full[r_base:r_base + 64, hrow0:hrow0 + 33].T.copy()  # [33,64]
        yhw = y[b, :, hrow0:hrow0 + 33, :].transpose(1, 0, 2).copy()  # [33,2,64]
        gmb = np.full((33, 1), gval, np.float32)

        in_maps.append({
            "xsh": xsh, "qfp": qfp, "kfp": kfp, "von": von, "wcd": wc,
            "s18": s18, "sht": np.ascontiguousarray(sht), "swt": swt,
            "yhw": np.ascontiguousarray(yhw), "gmb": gmb,
            "bna": bna, "bnb": bnb,
        })
    return in_maps


_CACHED_NC = None


def kernel(**inputs) -> np.ndarray:
    global _CACHED_NC
    in_maps = shard_inputs(**inputs)
    if _CACHED_NC is None:
        _CACHED_NC = build_program(repeat=1)
    res = run_bass_kernel_spmd(_CACHED_NC, in_maps, core_ids=list(range(8)))
    out = np.zeros((B, CY, H, W), np.float32)
    for core in range(8):
        b, half = core // 2, core % 2
        out[b, :, half * 64:half * 64 + 64, :] = res.results[core]["o"]
    return out


# revision 8
# speedup vs baseline: 1.2673x; 1.2673x over previous
"""Trainium2 Bass kernel for nn_FFU_87625922773299 (sparse_attention).

Self-contained: accepts FULL inputs, shards over 8 NeuronCores
(core = 2*b + half; each core does one sample's attention i-half plus the
matching conv/upsample/combine output rows), returns FULL [4,2,128,128].
"""
import numpy as np

import concourse.bass as bass
import concourse.mybir as mybir
import concourse.tile as tile
from concourse import bacc
from concourse.bass_utils import run_bass_kernel_spmd

f32 = mybir.dt.float32
f32r = mybir.dt.float32r
Act = mybir.ActivationFunctionType

B, CIN, H, W = 4, 1024, 128, 128
CY, HY, WY = 2, 64, 64
N = HY * WY
ISPAN = 2112           # per-core i-range (33 y-rows * 64)
I0S = [0, 1984]
NROW = 64              # output rows per core
NSLOT = 66             # x row-slots (64 rows + top/bottom halo-or-zero)
WP = 130               # padded width (guard cols)
RG = 6                 # conv row-group size
SUPERS = [(0, 1024), (1024, 1024), (2048, 64)]
NCHUNK = 8
PB = 1                 # P18 front guard
P18LEN = PB + NSLOT * WP + 1


def build_program(repeat=1):
    nc = bacc.Bacc(trn_type="TRN2")
    xsh = nc.dram_tensor("xsh", [CIN, NSLOT, WP], f32r, kind="ExternalInput")
    qfp = nc.dram_tensor("qfp", [128, ISPAN], f32r, kind="ExternalInput")
    kfp = nc.dram_tensor("kfp", [128, 32, 128], f32r, kind="ExternalInput")
    von = nc.dram_tensor("von", [128, 32, 3], mybir.dt.bfloat16, kind="ExternalInput")
    wcd = nc.dram_tensor("wcd", [128, NCHUNK, 18], f32r, kind="ExternalInput")
    s18 = nc.dram_tensor("s18", [128, 9, 2], f32r, kind="ExternalInput")
    sht = nc.dram_tensor("sht", [33, 64], f32r, kind="ExternalInput")
    swt = nc.dram_tensor("swt", [64, 128], f32r, kind="ExternalInput")
    yhw = nc.dram_tensor("yhw", [33, 2, 64], f32r, kind="ExternalInput")
    gmb = nc.dram_tensor("gmb", [33, 1], f32, kind="ExternalInput")
    bna = nc.dram_tensor("bna", [2, 1], f32, kind="ExternalInput")
    bnb = nc.dram_tensor("bnb", [2, 1], f32, kind="ExternalInput")
    o = nc.dram_tensor("o", [2, NROW, 128], f32, kind="ExternalOutput")

    with tile.TileContext(nc) as tc:
        with (
            tc.tile_pool(name="const", bufs=1) as cp,
            tc.tile_pool(name="xg", bufs=2) as xgp,
            tc.tile_pool(name="big", bufs=1) as bigp,
            tc.tile_pool(name="we", bufs=2) as wep,
            tc.tile_pool(name="sm", bufs=1) as smp,
        ):
            qf_sb = cp.tile([128, ISPAN], f32r)
            nc.sync.dma_start(out=qf_sb[:], in_=qfp[:])
            kf_sb = cp.tile([128, 32, 128], f32r)
            nc.sync.dma_start(out=kf_sb[:], in_=kfp[:])
            von_sb = cp.tile([128, 32, 3], mybir.dt.bfloat16)
            nc.sync.dma_start(out=von_sb[:], in_=von[:])
            wc_sb = cp.tile([128, NCHUNK, 18], f32r)
            nc.sync.dma_start(out=wc_sb[:], in_=wcd[:])
            s18_sb = cp.tile([128, 9, 2], f32r)
            nc.sync.dma_start(out=s18_sb[:], in_=s18[:])
            sht_sb = cp.tile([33, 64], f32r)
            nc.sync.dma_start(out=sht_sb[:], in_=sht[:])
            swt_sb = cp.tile([64, 128], f32r)
            nc.sync.dma_start(out=swt_sb[:], in_=swt[:])
            yhw_sb = cp.tile([33, 2, 64], f32r)
            nc.sync.dma_start(out=yhw_sb[:], in_=yhw[:])
            gmb_sb = cp.tile([33, 1], f32)
            nc.sync.dma_start(out=gmb_sb[:], in_=gmb[:])
            bna_sb = cp.tile([2, 1], f32)
            nc.sync.dma_start(out=bna_sb[:], in_=bna[:])
            bnb_sb = cp.tile([2, 1], f32)
            nc.sync.dma_start(out=bnb_sb[:], in_=bnb[:])

            p18_sb = bigp.tile([128, P18LEN], f32r)
            numer_sb = bigp.tile([3, ISPAN], f32)
            xc_sb = bigp.tile([2, NROW * 128], f32)

            # zero-fill p18 once (rows >=18 + guards stay zero forever)
            zsrc = cp.tile([128, 1], f32)
            nc.vector.memset(zsrc[:], 0.0)
            zbc = bass.AP(tensor=zsrc.tensor, offset=zsrc.offset,
                          ap=[zsrc.ap[0], [0, P18LEN]])
            nc.vector.tensor_copy(out=p18_sb[:], in_=zbc)

            def body(it):

                # ---- phase 1: attention + conv main (8 psum banks) ----
                with (
                    tc.tile_pool(name="eps", bufs=2, space="PSUM") as epsp,
                    tc.tile_pool(name="nps", bufs=1, space="PSUM") as npsp,
                    tc.tile_pool(name="cps", bufs=2, space="PSUM") as cpsp,
                ):
                    for (s0, span) in SUPERS:
                        nsub = (span + 511) // 512
                        n_ps = npsp.tile([3, span], f32, tag="numer",
                                         name=f"n_{it}_{s0}")
                        for g in range(32):
                            e_ps = epsp.tile([128, span], f32, tag="E",
                                             name=f"e_{it}_{s0}_{g}")
                            for t in range(nsub):
                                a, b2 = t * 512, min(span, (t + 1) * 512)
                                nc.tensor.matmul(
                                    e_ps[:, a:b2], kf_sb[:, g, :],
                                    qf_sb[:, s0 + a:s0 + b2],
                                    start=True, stop=True)
                            wexp = wep.tile([128, span], mybir.dt.bfloat16, tag="W",
                                            name=f"w_{it}_{s0}_{g}")
                            nc.scalar.activation(wexp[:], e_ps[:], Act.Exp)
                            for t in range(nsub):
                                a, b2 = t * 512, min(span, (t + 1) * 512)
                                nc.tensor.matmul(
                                    n_ps[:, a:b2], von_sb[:, g, :], wexp[:, a:b2],
                                    start=(g == 0), stop=(g == 31))
                        nc.vector.tensor_copy(out=numer_sb[:, s0:s0 + span],
                                              in_=n_ps[:])

                    ngrp = (NROW + RG - 1) // RG
                    for gi in range(ngrp):
                        r0 = gi * RG
                        nrows = min(RG, NROW - r0)
                        n_sl = nrows + 2
                        glen = n_sl * WP
                        xg = xgp.tile([128, NCHUNK, (RG + 2) * WP], f32r,
                                      tag="xg", name=f"xg_{it}_{gi}")
                        for ch in range(NCHUNK):
                            nc.sync.dma_start(
                                out=xg[:, ch, 0:glen],
                                in_=xsh[ch * 128:(ch + 1) * 128, r0:r0 + n_sl, :]
                                .rearrange("c s w -> c (s w)"))
                        nblk = (glen + 511) // 512
                        for t in range(nblk):
                            q0 = t * 512
                            qn = min(512, glen - q0)
                            c_ps = cpsp.tile([18, 512], f32, tag="P18",
                                             name=f"c_{it}_{gi}_{t}")
                            for ch in range(NCHUNK):
                                nc.tensor.matmul(
                                    c_ps[:18, 0:qn], wc_sb[:, ch, :],
                                    xg[:, ch, q0:q0 + qn],
                                    start=(ch == 0), stop=(ch == NCHUNK - 1))
                            dst = PB + r0 * WP + q0
                            nc.vector.tensor_copy(
                                out=p18_sb[0:18, dst:dst + qn],
                                in_=c_ps[:18, 0:qn])

                # ---- phase 2: tap-sum + BN/ReLU + tail (fresh psum) ----
                with (
                    tc.tile_pool(name="tps", bufs=2, space="PSUM") as tpsp,
                    tc.tile_pool(name="dscr", bufs=1, space="DRAM") as dsp,
                ):
                    olen = NROW * WP
                    nblk = (olen + 511) // 512
                    for t in range(nblk):
                        q0 = t * 512
                        qn = min(512, olen - q0)
                        t_ps = tpsp.tile([2, 512], f32, tag="TAP",
                                         name=f"t_{it}_{t}")
                        k = 0
                        for dy in range(3):
                            for dx in range(3):
                                off = PB + q0 + dy * WP + dx - 1
                                nc.tensor.matmul(
                                    t_ps[:2, 0:qn], s18_sb[:, dy * 3 + dx, :],
                                    p18_sb[:, off:off + qn],
                                    start=(k == 0), stop=(k == 8))
                                k += 1
                        r_lo, r_hi = q0 // WP, (q0 + qn - 1) // WP
                        for r in range(r_lo, r_hi + 1):
                            a = max(q0, r * WP + 1)
                            b2 = min(q0 + qn, r * WP + 129)
                            if a >= b2 or r >= NROW:
                                continue
                            w_lo = a - (r * WP + 1)
                            nc.scalar.activation(
                                out=xc_sb[:, r * 128 + w_lo:r * 128 + w_lo + (b2 - a)],
                                in_=t_ps[:2, a - q0:b2 - q0],
                                func=Act.Relu, bias=bnb_sb[:], scale=bna_sb[:])

                    # attention numer -> h-part layout (via DRAM scratch)
                    ndr = dsp.tile([3, ISPAN], f32, tag="ndr", name=f"ndr_{it}")
                    nc.sync.dma_start(out=ndr[:], in_=numer_sb[:])
                    nm_hw = smp.tile([33, 3, 64], f32, tag="nmhw", name=f"nm_{it}")
                    nc.sync.dma_start(
                        out=nm_hw[:],
                        in_=ndr.rearrange("c (h w) -> h c w", h=33))
                    rcp = smp.tile([33, 1, 64], f32, tag="rcp", name=f"r_{it}")
                    nc.vector.reciprocal(out=rcp[:], in_=nm_hw[:, 0:1, :])
                    att = smp.tile([33, 2, 64], f32, tag="att", name=f"a_{it}")
                    for c in range(2):
                        nc.vector.tensor_mul(out=att[:, c, :],
                                             in0=nm_hw[:, c + 1, :],
                                             in1=rcp[:, 0, :])
                    y1hw = smp.tile([33, 2, 64], f32r, tag="y1hw", name=f"y1_{it}")
                    tmp = smp.tile([33, 2, 64], f32, tag="tmpy", name=f"ty_{it}")
                    nc.vector.tensor_scalar_mul(tmp[:], att[:], gmb_sb[:])
                    nc.vector.tensor_add(out=y1hw[:], in0=tmp[:], in1=yhw_sb[:])

                    # upsample y and y1 -> [w' 128, h' 64] per channel
                    ups = []
                    for (src, nm) in ((yhw_sb, "uy"), (y1hw, "uy1")):
                        for c in range(2):
                            t1p = tpsp.tile([64, 64], f32, tag="T1",
                                            name=f"t1_{it}_{nm}_{c}")
                            nc.tensor.matmul(t1p[:], src[:, c, :], sht_sb[:],
                                             start=True, stop=True)
                            t1s = smp.tile([64, 64], f32r, tag="T1s",
                                           name=f"t1s_{it}_{nm}_{c}")
                            nc.vector.tensor_copy(out=t1s[:], in_=t1p[:])
                            up = tpsp.tile([128, 64], f32, tag="UP",
                                           name=f"up_{it}_{nm}_{c}")
                            nc.tensor.matmul(up[:], swt_sb[:], t1s[:],
                                             start=True, stop=True)
                            us = smp.tile([128, 64], f32, tag=f"us{nm}{c}",
                                          name=f"us_{it}_{nm}_{c}")
                            nc.vector.tensor_copy(out=us[:], in_=up[:])
                            ups.append(us)
                    yup0, yup1, y1up0, y1up1 = ups

                    # xc [2, r*128+w] -> xcT [w 128, r, c] via scatter DMA
                    xdr = dsp.tile([2, NROW, 128], f32, tag="xdr", name=f"xdr_{it}")
                    nc.sync.dma_start(out=xdr.rearrange("c r w -> c (r w)"),
                                      in_=xc_sb[:])
                    xcT = smp.tile([128, 2, NROW], f32, tag="xcT",
                                   name=f"xct_{it}")
                    nc.sync.dma_start(out=xcT[:],
                                      in_=xdr.rearrange("c r w -> w c r"))

                    comb = smp.tile([128, 2, NROW], f32, tag="comb",
                                    name=f"cb_{it}")
                    for c, (yu, y1u) in enumerate(((yup0, y1up0),
                                                   (yup1, y1up1))):
                        nc.vector.tensor_mul(out=comb[:, c, :],
                                             in0=xcT[:, c, :], in1=y1u[:])
                        nc.vector.tensor_add(out=comb[:, c, :],
                                             in0=comb[:, c, :], in1=yu[:])
                    for c in range(2):
                        nc.sync.dma_start(out=o[c].rearrange("r w -> w r"),
                                          in_=comb[:, c, :])

            if repeat == 1:
                body(0)
            else:
                with tc.For_i(0, repeat):
                    body(0)
    nc.compile()
    return nc


# ---------------- host side ----------------

def _upsample_matrix(out_n, in_n):
    s = np.zeros((out_n, in_n), np.float32)
    pos = np.arange(out_n) * (in_n - 1.0) / (out_n - 1.0)
    lo = np.floor(pos).astype(np.int64)
    hi = np.minimum(lo + 1, in_n - 1)
    w = (pos - lo).astype(np.float32)
    s[np.arange(out_n), lo] += 1.0 - w
    s[np.arange(out_n), hi] += w
    return s


def shard_inputs(x, y, wq, bq, wk, bk, wv, bv, gamma, conv_w, conv_b,
                 bn_scale, bn_bias, bn_mean, bn_var):
    x = np.asarray(x, np.float32)
    y = np.asarray(y, np.float32)
    yf = y.reshape(B, CY, N)
    qf = np.einsum("oc,bcn->bon", np.asarray(wq, np.float32), yf) + \
        np.asarray(bq, np.float32)[None, :, None]
    kf = np.einsum("oc,bcn->bon", np.asarray(wk, np.float32), yf) + \
        np.asarray(bk, np.float32)[None, :, None]
    vf = np.einsum("oc,bcn->bon", np.asarray(wv, np.float32), yf) + \
        np.asarray(bv, np.float32)[None, :, None]

    # conv weights: [128 cin_in_chunk, chunk, (dy,dx,co)]
    cw = np.asarray(conv_w, np.float32)  # [2, 1024, 3, 3]
    wc = np.zeros((128, NCHUNK, 18), np.float32)
    for ch in range(NCHUNK):
        blk = cw[:, ch * 128:(ch + 1) * 128, :, :]  # [2,128,3,3]
        wc[:, ch, :] = blk.transpose(1, 2, 3, 0).reshape(128, 18)

    s18 = np.zeros((128, 9, 2), np.float32)
    for dy in range(3):
        for dx in range(3):
            for co in range(2):
                s18[dy * 6 + dx * 2 + co, dy * 3 + dx, co] = 1.0

    sh_full = _upsample_matrix(H, HY)     # [128, 64]
    sw = _upsample_matrix(W, WY)          # [128, 64]
    swt = sw.T.copy()                     # [64, 128] SwT[w, w'] = Sw[w', w]

    inv = np.asarray(bn_scale, np.float32) / np.sqrt(
        np.asarray(bn_var, np.float32) + 1e-5)
    bna = inv.reshape(2, 1)
    bnb = ((np.asarray(conv_b, np.float32) - np.asarray(bn_mean, np.float32))
           * inv + np.asarray(bn_bias, np.float32)).reshape(2, 1)
    gval = float(np.asarray(gamma).reshape(-1)[0])

    in_maps = []
    for core in range(8):
        b, half = core // 2, core % 2
        i0 = I0S[half]
        r_base = half * 64

        xsh = np.zeros((CIN, NSLOT, WP), np.float32)
        lo_row = r_base - 1
        for s in range(NSLOT):
            row = lo_row + s
            if 0 <= row < H:
                xsh[:, s, 1:129] = x[b, :, row, :]

        qfp = np.zeros((128, ISPAN), np.float32)
        qfp[0:2] = qf[b, :, i0:i0 + ISPAN]
        kfp = np.zeros((128, 32, 128), np.float32)
        kfp[0:2] = kf[b].reshape(2, 32, 128)
        import ml_dtypes
        von = np.zeros((128, 32, 3), ml_dtypes.bfloat16)
        von[:, :, 0] = 1.0
        von[:, :, 1] = vf[b, 0].reshape(32, 128).T
        von[:, :, 2] = vf[b, 1].reshape(32, 128).T

        hrow0 = 31 * half
        sht = sh_full[r_base:r_base + 64, hrow0:hrow0 + 33].T.copy()  # [33,64]
        yhw = y[b, :, hrow0:hrow0 + 33, :].transpose(1, 0, 2).copy()  # [33,2,64]
        gmb = np.full((33, 1), gval, np.float32)

        in_maps.append({
            "xsh": xsh, "qfp": qfp, "kfp": kfp, "von": von, "wcd": wc,
            "s18": s18, "sht": np.ascontiguousarray(sht), "swt": swt,
            "yhw": np.ascontiguousarray(yhw), "gmb": gmb,
            "bna": bna, "bnb": bnb,
        })
    return in_maps


_CACHED_NC = None


def kernel(**inputs) -> np.ndarray:
    global _CACHED_NC
    in_maps = shard_inputs(**inputs)
    if _CACHED_NC is None:
        _CACHED_NC = build_program(repeat=1)
    res = run_bass_kernel_spmd(_CACHED_NC, in_maps, core_ids=list(range(8)))
    out = np.zeros((B, CY, H, W), np.float32)
    for core in range(8):
        b, half = core // 2, core % 2
        out[b, :, half * 64:half * 64 + 64, :] = res.results[core]["o"]
    return out
